# revision 64
# baseline (speedup 1.0000x reference)
"""Trainium2 Bass kernel for nn_NnBoard768 (sparse embedding-lookup NNUE head).

The bottleneck of any dma_gather formulation here is SWDGE descriptor
generation on the Pool engine: ~2.26us per 1024-descriptor gather (the HW
cap per instruction), serial across all queues.  Everything below exists
to minimize the number of gather instructions (and keep the Pool stream
gap-free); DMA/HBM, PE and DVE all run far below their limits.

Strategy (data-parallel over batch, 8 cores, input-specialized compile):
  - kernel() sees the actual indices, so the program + table layout are
    specialized per input (compile cached on the structural shape).
  - Batch rows are assigned to cores by greedy swaps so every core
    references <= ~32.7k unique table rows; the per-core table is REMAPPED
    to exactly those rows (fp8 e4m3, premultiplied by TSCALE=64) so every
    device id fits the int16 range of `dma_gather` in a single pass —
    no two-pass zero-row-junk scheme, every descriptor fetches a needed row.
  - RUN descriptors: each (position, side) unit owns a run of RROWS=14
    consecutive table rows (its claimed rows under a hash-randomized
    first-claim plus a steal pass that tops up units short of 14; residual
    deficits duplicate the unit's own draws).  ONE descriptor with
    elem_size=14*512B / elem_step=512B (an overlapping-stride table view)
    fetches the whole run, covering 14 of the unit's 32 draws; the other 18
    are plain 512B singles sorted ascending for HBM locality (16 in per-jh
    full gathers, the last 2 slots of all 4 jh merged into one per-phase
    gather).  Descriptors drop from 65536 to 38912 per core and gather
    instructions from 64 to 40; run gathers are 512 descriptors each so
    their ring drain mostly hides under the 4-queue rotation period.
  - Accumulation over the 32 active features runs on the tensor engine with
    fp8 DoubleRow matmuls (2 row-blocks per instruction, 0.5 cyc/row)
    against a stacked identity scaled by 1/TSCALE; b_ft is added by one
    bf16 identity matmul into the same PSUM accumulation group.
  - Work is split into 16 PSUM rounds (side x batch-half x jh block, one
    bank each, double-buffered); each gather covers a single jh block, so
    every bank's DVE epilogue (clip to [0,1], multiply by W_out, reduce)
    starts as soon as its own gathers land and hides under later gathers.
"""

import sys

sys.path.insert(0, "/opt/trn_rl_repo")

import numpy as np
import ml_dtypes

from concourse import bacc, bass, mybir
import concourse.tile as tile
from concourse.bass_utils import run_bass_kernel_spmd

P = 128          # SBUF partitions
K = 32           # nnz (active features per position)
J = 8            # batch slots per partition per core
JH = 4           # j-blocks per PSUM phase
F = 512          # feature-table output width
NCORES = 8
BPC = P * J      # batch rows per core (1024)
FT_IN = 40960

BBASE = 32768            # overflow ("B") region base device row
BCAP = 576               # max overflow rows
BZ0 = 576                # B-junk ids 576..639 (zero rows)
VDEV_B = BBASE + BCAP + 64  # device rows when an overflow region is needed

f32 = mybir.dt.float32
bf16 = mybir.dt.bfloat16
i16 = mybir.dt.int16
f8 = mybir.dt.float8e4
F8_NP = ml_dtypes.float8_e4m3
BF16_NP = ml_dtypes.bfloat16
Alu = mybir.AluOpType
DR = mybir.MatmulPerfMode.DoubleRow

TSCALE = 64.0    # host premultiplier; PE identity = 1/TSCALE
NQ = 4           # SWDGE descriptor-generation queues
GBUFS = 8
NPH = 4          # phases: (side, half)
KPG = 8          # k-slots per A-gather; covers ONE jh block of 128 positions
NGJ = K // KPG   # A-gathers per (phase, jh) = 4
NGA = NGJ * JH   # A-gathers per phase = 16
NIDX = KPG * P              # idxs per A-gather (1024-descriptor HW cap)
SA16 = NIDX // 16           # 64 idx cols per A-gather

GBUFS_A = 16     # ring depth for the 4KB/partition A-gather tiles
RROWS = 14       # claimed rows per (position, side) unit (one run descriptor)
NSING = K - RROWS           # single draws per unit (18)
SING_FULL = [(0, 8), (8, 16)]   # full 1024-desc single-gathers per (ph, jh)
TAIL0 = 16       # tail slots 16..17 go in ONE per-phase gather (4 jh x 2)
NSG_P = 2        # full single-gathers per (phase, jh) in paired mode


def _build(nbslots: int, paired: bool):
    nc = bacc.Bacc("TRN2", target_bir_lowering=False, debug=False,
                   num_devices=NCORES, num_swdge_queues=NQ)

    nsg = NSG_P if paired else NGJ     # full single-gathers per (phase, jh)
    ncols = (NSG_P * JH + 1) if paired else (NGJ * JH)
    vdev = VDEV_B if nbslots else BBASE
    wft = nc.dram_tensor("w_ft", [vdev, F], f8, kind="ExternalInput")
    idxa_in = [
        nc.dram_tensor(f"idxa{ph}", [P, ncols, SA16], i16,
                       kind="ExternalInput")
        for ph in range(NPH)
    ]
    if paired:
        idxp_in = nc.dram_tensor("idxp", [P, NPH, SA16 // 2], i16,
                                 kind="ExternalInput")
    if nbslots:
        idxb_in = nc.dram_tensor("idxb", [P, NPH, JH, nbslots * 8], i16,
                                 kind="ExternalInput")
    bias_in = nc.dram_tensor("bias", [P, F], bf16, kind="ExternalInput")
    wout_in = nc.dram_tensor("wout", [P, 2, F], bf16, kind="ExternalInput")
    bout_in = nc.dram_tensor("bout", [P, 1], f32, kind="ExternalInput")
    idw_in = nc.dram_tensor("idw", [P, 2, P], f8, kind="ExternalInput")
    idb_in = nc.dram_tensor("idb", [P, P], bf16, kind="ExternalInput")
    out = nc.dram_tensor("out", [P, J], f32, kind="ExternalOutput")

    qn = 0
    with tile.TileContext(nc) as tc:
        with tc.tile_pool(name="sbuf", bufs=1) as pool, \
             tc.tile_pool(name="gather", bufs=GBUFS) as gpool, \
             tc.tile_pool(name="psum", bufs=2, space="PSUM") as ppool:
            idxa = []
            for ph in range(NPH):
                t = pool.tile([P, ncols, SA16], i16, tag=f"idxa{ph}",
                              name=f"idxa{ph}")
                idxa.append(t)
            if paired:
                idxp = pool.tile([P, NPH, SA16 // 2], i16, tag="idxp")
                nc.sync.dma_start(out=idxp[:], in_=idxp_in[:])
            # phase-0 indices first so gathers can start immediately
            nc.sync.dma_start(out=idxa[0][:], in_=idxa_in[0][:])
            identW = pool.tile([P, 2, P], f8, tag="identW")
            nc.sync.dma_start(out=identW[:], in_=idw_in[:])
            for ph in range(1, NPH):
                nc.sync.dma_start(out=idxa[ph][:], in_=idxa_in[ph][:])
            if nbslots:
                idxb = pool.tile([P, NPH, JH, nbslots * 8], i16, tag="idxb")
                nc.sync.dma_start(out=idxb[:], in_=idxb_in[:])
            bias_sb = pool.tile([P, F], bf16, tag="bias")
            nc.sync.dma_start(out=bias_sb[:], in_=bias_in[:])
            wout_sb = pool.tile([P, 2, F], bf16, tag="wout")
            nc.sync.dma_start(out=wout_sb[:], in_=wout_in[:])
            bout_sb = pool.tile([P, 1], f32, tag="bout")
            nc.sync.dma_start(out=bout_sb[:], in_=bout_in[:])
            identB = pool.tile([P, P], bf16, tag="identB")
            nc.sync.dma_start(out=identB[:], in_=idb_in[:])

            z = [pool.tile([P, J], f32, tag=f"z{s}", name=f"z{s}")
                 for s in range(2)]
            prod = pool.tile([P, F], bf16, tag="prod")

            # run-gather view: one descriptor fetches a unit's whole
            # RROWS-row claimed run (elem_step=512 scales idx by single rows)
            run_view = bass.AP(wft, 0, [[F, vdev - RROWS + 1], [1, RROWS * F]])
            gq = None
            for ph in range(NPH):
                s, hf = divmod(ph, 2)
                qn += 1    # shift rotation so run-gather drains spread queues
                if paired:
                    gq = gpool.tile([P, JH, RROWS * F], f8, tag="gq",
                                    name=f"gq{ph}", bufs=2)
                    nc.gpsimd.dma_gather(
                        gq[:], run_view, idxp[:, ph, :],
                        num_idxs=NIDX // 2, num_idxs_reg=NIDX // 2,
                        elem_size=RROWS * F, elem_step=F, queue_num=qn % NQ)
                    qn += 1
                    # tail singles for all 4 jh of this phase in one gather
                    gt = gpool.tile([P, KPG, F], f8, tag="gt",
                                    name=f"gt{ph}", bufs=2)
                    nc.gpsimd.dma_gather(
                        gt[:], wft[:, :], idxa[ph][:, NSG_P * JH, :],
                        num_idxs=NIDX, num_idxs_reg=NIDX,
                        elem_size=F, queue_num=qn % NQ)
                    qn += 1
                for jh in range(JH):
                    acc = ppool.tile([P, F], f32, tag=f"acc{jh}",
                                     name=f"acc{ph}_{jh}")
                    first = True
                    if paired:
                        rhs8 = gq[:, jh, :].rearrange(
                            "p (i f) -> p i f", i=RROWS)
                        for sl in range(RROWS // 2):
                            nc.tensor.matmul(
                                acc[:], identW[:], rhs8[:, 2 * sl:2 * sl + 2, :],
                                start=first, stop=False, perf_mode=DR)
                            first = False
                        if RROWS % 2:
                            nc.tensor.matmul(
                                acc[:], identW[:, 0, :], rhs8[:, RROWS - 1, :],
                                start=first, stop=False)
                            first = False
                    chunks = (SING_FULL if paired
                              else [(KPG * g, KPG * (g + 1))
                                    for g in range(nsg)])
                    for g, (k0, k1) in enumerate(chunks):
                        w = k1 - k0
                        ga = gpool.tile([P, w, F], f8, tag=f"ga{w}",
                                        name=f"ga{ph}_{jh}_{g}", bufs=GBUFS_A)
                        nc.gpsimd.dma_gather(
                            ga[:], wft[:, :],
                            idxa[ph][:, jh * nsg + g, 0:w * 8],
                            num_idxs=w * P, num_idxs_reg=w * P,
                            elem_size=F, queue_num=qn % NQ)
                        qn += 1
                        for kk in range(w // 2):
                            nc.tensor.matmul(
                                acc[:], identW[:],
                                ga[:, 2 * kk:2 * kk + 2, :],
                                start=first, stop=False,
                                perf_mode=DR)
                            first = False
                        if w % 2:
                            nc.tensor.matmul(
                                acc[:], identW[:, 0, :], ga[:, w - 1, :],
                                start=first, stop=False)
                            first = False
                    if paired:
                        nc.tensor.matmul(
                            acc[:], identW[:], gt[:, 2 * jh:2 * jh + 2, :],
                            start=first, stop=False, perf_mode=DR)
                        first = False
                    if nbslots:
                        gb = gpool.tile([P, nbslots, F], f8, tag=f"gb{jh}",
                                        name=f"gb{ph}_{jh}", bufs=2)
                        nc.gpsimd.dma_gather(
                            gb[:], wft[BBASE:, :], idxb[:, ph, jh, :],
                            num_idxs=nbslots * P, num_idxs_reg=nbslots * P,
                            elem_size=F, queue_num=qn % NQ)
                        qn += 1
                        for m in range(nbslots):
                            nc.tensor.matmul(
                                acc[:], identW[:, 0, :], gb[:, m, :],
                                start=False, stop=False)
                    nc.tensor.matmul(acc[:], identB[:], bias_sb[:],
                                     start=False, stop=True)

                    h = pool.tile([P, F], bf16, tag=f"h{jh}",
                                  name=f"h{ph}_{jh}")
                    nc.vector.tensor_scalar(
                        out=h[:], in0=acc[:], scalar1=0.0, scalar2=1.0,
                        op0=Alu.max, op1=Alu.min)
                    nc.vector.tensor_tensor(out=prod[:], in0=h[:],
                                            in1=wout_sb[:, s, :],
                                            op=Alu.mult)
                    nc.vector.tensor_reduce(
                        out=z[s][:, hf * JH + jh: hf * JH + jh + 1],
                        in_=prod[:], axis=mybir.AxisListType.X, op=Alu.add)

            nc.vector.tensor_tensor(out=z[0][:], in0=z[0][:], in1=z[1][:],
                                    op=Alu.add)
            out_sb = pool.tile([P, J], f32, tag="out")
            nc.scalar.activation(
                out=out_sb[:], in_=z[0][:],
                func=mybir.ActivationFunctionType.Sigmoid,
                bias=bout_sb[:, :1])
            nc.sync.dma_start(out=out.ap(), in_=out_sb[:])

    nc.compile()
    return nc


_cache = {}


def _get(nbslots: int, paired: bool):
    key = (nbslots, paired)
    if key not in _cache:
        _cache[key] = _build(nbslots, paired)
    return _cache[key]


def _wrap16(vec: np.ndarray) -> np.ndarray:
    """int vector -> [128, len//16] int16 tile (idx i at partition i%16,
    col i//16, replicated across the 8 16-partition groups)."""
    arr = vec.reshape(-1, 16).T.astype(np.int16)
    return np.tile(arr, (8, 1))


def _rebalance(stm: np.ndarray, nstm: np.ndarray):
    """Assign batch rows to cores so every core references <= 32768 unique
    table rows (then int16 gather ids cover them directly, no overflow
    pass).  Greedy pair swaps between the worst and best cores."""
    target = BBASE - 48      # headroom for paired-mode deficit duplicates
    draws = np.concatenate([stm, nstm], axis=1).astype(np.int64)  # [8192, 64]
    rows = np.arange(NCORES * BPC).reshape(NCORES, BPC)
    cnts = [np.bincount(draws[rows[c]].ravel(), minlength=FT_IN)
            for c in range(NCORES)]
    for _ in range(256):
        us = np.array([(c > 0).sum() for c in cnts])
        if us.max() <= target:
            return rows, True
        cmax, cmin = int(us.argmax()), int(us.argmin())
        dmax = draws[rows[cmax]]
        excl = (cnts[cmax][dmax] == 1).sum(axis=1)
        ra = int(excl.argmax())
        dmin = draws[rows[cmin]]
        newu = (cnts[cmax][dmin] == 0).sum(axis=1)
        rb = int(newu.argmin())
        ga, gb = rows[cmax][ra], rows[cmin][rb]
        np.subtract.at(cnts[cmax], draws[ga], 1)
        np.add.at(cnts[cmax], draws[gb], 1)
        np.subtract.at(cnts[cmin], draws[gb], 1)
        np.add.at(cnts[cmin], draws[ga], 1)
        rows[cmax][ra], rows[cmin][rb] = gb, ga
    us = np.array([(c > 0).sum() for c in cnts])
    return rows, bool(us.max() <= BBASE)


def _prep_core(stm_c: np.ndarray, nstm_c: np.ndarray, W_ft: np.ndarray,
               cap: int):
    """Remap one core's referenced table rows into the device layout and
    build its gather index vectors.

    Returns (w_dev fp8 table, avecs (NPH*NGA vectors of NIDX ids),
    bvecs (per phase: per-overflow-slot 512-id vectors), maxb)."""
    both = np.stack([stm_c, nstm_c]).astype(np.int64)   # [2, 1024, 32]
    counts = np.bincount(both.ravel(), minlength=FT_IN)
    usedrows = np.flatnonzero(counts)
    u = usedrows.size
    e = max(0, u - cap)
    if e:
        order = np.argsort(counts[usedrows], kind="stable")
        excess_rows = np.sort(usedrows[order[:e]])
        a_rows = np.sort(usedrows[order[e:]])
    else:
        excess_rows = np.empty(0, np.int64)
        a_rows = usedrows
    assert e <= BCAP, f"overflow region too small: {e} > {BCAP}"

    remap = np.zeros(FT_IN, np.int64)
    remap[a_rows] = np.arange(a_rows.size)
    remap[excess_rows] = BBASE + np.arange(e)
    dev = remap[both]                                   # [2, 1024, 32]
    dev.sort(axis=2)      # ascending: banded A ids first, overflow at tail

    vdev = VDEV_B if e else BBASE
    w_dev = np.zeros((vdev, F), dtype=F8_NP)
    w_dev[:a_rows.size] = (W_ft[a_rows] * TSCALE).astype(F8_NP)
    if e:
        w_dev[BBASE:BBASE + e] = (W_ft[excess_rows] * TSCALE).astype(F8_NP)

    spreadA = cap + (np.arange(NIDX) * 7) % 64
    spread128 = BZ0 + (np.arange(P) * 7) % 64
    avecs = []     # NPH * NGA vectors of NIDX ids, ordered (ph, jh, g)
    bvecs = []     # [ph][jh][m] -> 128-id vector (B-region-relative)
    maxb = 0
    for ph in range(NPH):
        s, hf = divmod(ph, 2)
        seg = dev[s, hf * JH * P:(hf + 1) * JH * P, :]   # [512, 32]
        segr = seg.reshape(JH, P, K)     # [jh, p, k]
        for jh in range(JH):
            for g in range(NGJ):
                # descriptor b*128+p -> (k = KPG*g + b, jh, p)
                vec = segr[jh, :, KPG * g:KPG * (g + 1)].T.ravel()
                vec = np.where(vec >= BBASE, spreadA, vec)
                avecs.append(vec)
        nB = (segr >= BBASE).sum(axis=2)                 # [jh, p]
        maxb = max(maxb, int(nB.max()))
        cols = [
            [np.where(nB[jh] > m,
                      segr[jh, np.arange(P), K - 1 - m] - BBASE, spread128)
             for m in range(int(nB.max()))]
            for jh in range(JH)
        ]
        bvecs.append(cols)
    return w_dev, avecs, bvecs, maxb


def _prep_core_paired(stm_c: np.ndarray, nstm_c: np.ndarray,
                      W_ft: np.ndarray):
    """Paired layout: each (position, side) unit owns a run of 8 table rows
    (its first-claimed rows, deficit filled by duplicating its own draws) so
    4 descriptors of elem_size=1024/elem_step=512 cover 8 of its 32 draws;
    the remaining 24 draws are plain 512B singles.

    Returns (w_dev, avecs [ph][jh*3+g] single vectors, pvecs [ph][pg] pair
    first-id vectors, ok)."""
    draws = np.stack([stm_c, nstm_c]).astype(np.int64)   # [2, 1024, 32]
    flat = draws.reshape(2 * BPC, K)                     # unit-major
    NU = 2 * BPC
    # hash-randomized claim priority so expected claims are uniform per unit
    rflat = flat.ravel()
    uflat = np.repeat(np.arange(NU), K)
    keys = (uflat * 2654435761 + rflat * 40503) % (1 << 31)
    best = np.full(FT_IN, 1 << 62, np.int64)
    np.minimum.at(best, rflat, keys)
    claim = np.full(FT_IN, NU, np.int64)
    winner = keys == best[rflat]
    np.minimum.at(claim, rflat[winner], uflat[winner])
    used = np.flatnonzero(np.bincount(rflat, minlength=FT_IN) > 0)

    RUN = RROWS
    # steal pass: units short of RUN claims take rows they draw from
    # owners holding more than RUN
    cl_count = np.bincount(claim[used], minlength=NU + 1)[:NU]
    for u in np.flatnonzero(cl_count < RUN):
        need = RUN - cl_count[u]
        for r in np.unique(flat[u]):
            if need <= 0:
                break
            o = claim[r]
            if o != u and o < NU and cl_count[o] > RUN:
                claim[r] = u
                cl_count[o] -= 1
                cl_count[u] += 1
                need -= 1

    order = np.lexsort((used, claim[used]))
    cl_sorted = used[order]
    cl_unit = claim[used][order]
    starts = np.searchsorted(cl_unit, np.arange(NU))
    ends = np.searchsorted(cl_unit, np.arange(NU) + 1)
    pair_rows = np.empty((NU, RUN), np.int64)
    remap = np.full(FT_IN, -1, np.int64)
    for u in range(NU):
        rows_u = cl_sorted[starts[u]:min(ends[u], starts[u] + RUN)]
        if rows_u.size < RUN:
            extra = np.setdiff1d(np.unique(flat[u]), rows_u)
            need = RUN - rows_u.size
            if extra.size < need:
                return None, None, None, False
            rows_u = np.concatenate([rows_u, extra[:need]])
        pair_rows[u] = rows_u
        nclaim = min(ends[u], starts[u] + RUN) - starts[u]
        remap[rows_u[:nclaim]] = u * RUN + np.arange(nclaim)
    general = used[remap[used] < 0]                      # sorted ascending
    base = NU * RUN
    if base + general.size > BBASE:
        return None, None, None, False
    remap[general] = base + np.arange(general.size)

    w_dev = np.zeros((BBASE, F), dtype=F8_NP)
    w_dev[:base] = (W_ft[pair_rows.ravel()] * TSCALE).astype(F8_NP)
    w_dev[base:base + general.size] = (
        W_ft[general] * TSCALE).astype(F8_NP)

    # per unit: the NSING single draws (draws minus one per run row)
    singles = np.empty((NU, NSING), np.int64)
    for u in range(NU):
        rows, cnts = np.unique(flat[u], return_counts=True)
        cover = np.isin(rows, pair_rows[u]).astype(np.int64)
        rest = np.repeat(rows, cnts - cover)
        if rest.size != NSING:
            return None, None, None, False
        singles[u] = np.sort(remap[rest])
    singles = singles.reshape(2, J, P, NSING)            # [s, j, p, slot]

    avecs = []
    for ph in range(NPH):
        s, hf = divmod(ph, 2)
        av = []
        for jh in range(JH):
            sl = singles[s, hf * JH + jh]                # [p, NSING]
            for k0, k1 in SING_FULL:
                av.append(sl[:, k0:k1].T.ravel())
        # combined tail gather: block b -> (jh = b//2, slot = TAIL0 + b%2)
        tail = np.empty(NIDX, np.int64)
        for b in range(KPG):
            tail[b * P:(b + 1) * P] =                 singles[s, hf * JH + b // 2, :, TAIL0 + b % 2]
        av.append(tail)
        avecs.append(av)
    # run-gather ids: desc i of phase (s, hf) covers the 8-row run of unit
    # s*BPC + hf*512 + i
    pvecs = [(s * BPC + hf * (JH * P) + np.arange(JH * P)) * RUN
             for s in (0, 1) for hf in (0, 1)]
    return w_dev, avecs, pvecs, True


def _kernel_np(stm_indices, nstm_indices, values, W_ft, b_ft, W_out, b_out):
    """Correct fallback for inputs the HW fast path doesn't cover."""
    stm_ft = np.einsum("bk,bkf->bf", values, W_ft[stm_indices]) + b_ft
    nstm_ft = np.einsum("bk,bkf->bf", values, W_ft[nstm_indices]) + b_ft
    hidden = np.clip(np.concatenate([stm_ft, nstm_ft], axis=1), 0.0, 1.0)
    return 1.0 / (1.0 + np.exp(-(hidden @ W_out + b_out)))


def kernel(stm_indices, nstm_indices, values, W_ft, b_ft, W_out, b_out,
           _trace=False):
    stm_indices = np.asarray(stm_indices)
    nstm_indices = np.asarray(nstm_indices)
    values = np.asarray(values, dtype=np.float32)
    W_ft = np.ascontiguousarray(np.asarray(W_ft, dtype=np.float32))
    b_ft = np.asarray(b_ft, dtype=np.float32)
    W_out = np.asarray(W_out, dtype=np.float32)
    b_out = np.asarray(b_out, dtype=np.float32)

    if not np.all(values == 1.0):
        r = _kernel_np(stm_indices, nstm_indices, values, W_ft, b_ft,
                       W_out, b_out)
        return (r, None) if _trace else r

    rows8, balanced = _rebalance(stm_indices, nstm_indices)
    paired = balanced
    preps = []
    nbslots = 0
    if paired:
        for c in range(NCORES):
            rc = rows8[c]
            w_dev, avecs, pvecs, ok = _prep_core_paired(
                stm_indices[rc], nstm_indices[rc], W_ft)
            if not ok:
                paired = False
                preps = []
                break
            preps.append((w_dev, avecs, pvecs))
    if not paired:
        cap = BBASE if balanced else BBASE - 64
        for c in range(NCORES):
            rc = rows8[c]
            p = _prep_core(stm_indices[rc], nstm_indices[rc], W_ft, cap)
            nbslots = max(nbslots, p[3])
            preps.append(p)

    nc = _get(nbslots, paired)
    vdev = VDEV_B if nbslots else BBASE

    bias_rep = np.ascontiguousarray(
        np.broadcast_to(b_ft, (P, F)).astype(BF16_NP))
    wout_rep = np.ascontiguousarray(
        np.broadcast_to(W_out[:, 0].reshape(2, F)[None, :, :],
                        (P, 2, F)).astype(BF16_NP))
    bout_rep = np.full((P, 1), b_out[0], dtype=np.float32)
    idw = np.zeros((P, 2, P), dtype=F8_NP)
    idw[:, 0, :] = idw[:, 1, :] = (np.eye(P) / TSCALE).astype(F8_NP)
    idb = np.ascontiguousarray(np.eye(P).astype(BF16_NP))

    in_maps = []
    for c in range(NCORES):
        if paired:
            w_dev, avecs, pvecs = preps[c]
            bvecs = None
        else:
            w_dev, avecs, bvecs, _ = preps[c]
        if w_dev.shape[0] != vdev:
            w_full = np.zeros((vdev, F), dtype=F8_NP)
            w_full[:w_dev.shape[0]] = w_dev
            w_dev = w_full
        m = {
            "w_ft": w_dev,
            "bias": bias_rep,
            "wout": wout_rep,
            "bout": bout_rep,
            "idw": idw,
            "idb": idb,
        }
        if paired:
            for ph in range(NPH):
                m[f"idxa{ph}"] = np.ascontiguousarray(np.stack(
                    [_wrap16(v) for v in avecs[ph]], axis=1))
            m["idxp"] = np.ascontiguousarray(np.stack(
                [_wrap16(v) for v in pvecs], axis=1))
        else:
            for ph in range(NPH):
                m[f"idxa{ph}"] = np.ascontiguousarray(np.stack(
                    [_wrap16(v) for v in avecs[ph * NGA:(ph + 1) * NGA]],
                    axis=1))
        if nbslots:
            idxb = np.zeros((P, NPH, JH, nbslots * 8), np.int16)
            for ph in range(NPH):
                for jh in range(JH):
                    for mi in range(nbslots):
                        if mi < len(bvecs[ph][jh]):
                            vec = bvecs[ph][jh][mi]
                        else:
                            vec = BZ0 + (np.arange(P) * 7) % 64
                        idxb[:, ph, jh, mi * 8:(mi + 1) * 8] = _wrap16(vec)
            m["idxb"] = np.ascontiguousarray(idxb)
        in_maps.append(m)

    res = run_bass_kernel_spmd(
        nc, in_maps, core_ids=list(range(NCORES)), trace=_trace
    )
    # core c's position j*128 + p holds batch row rows8[c][j*128 + p]
    out = np.zeros((NCORES * BPC, 1), dtype=np.float32)
    for c in range(NCORES):
        out[rows8[c], 0] = res.results[c]["out"].T.reshape(BPC)
    if _trace:
        return out, res
    return out


# revision 65
# speedup vs baseline: 1.0834x; 1.0834x over previous
"""Trainium2 Bass kernel for nn_NnBoard768 (sparse embedding-lookup NNUE head).

The bottleneck of any dma_gather formulation here is SWDGE descriptor
generation on the Pool engine: ~2.26us per 1024-descriptor gather (the HW
cap per instruction), serial across all queues.  Everything below exists
to minimize the number of gather instructions (and keep the Pool stream
gap-free); DMA/HBM, PE and DVE all run far below their limits.

Strategy (data-parallel over batch, 8 cores, input-specialized compile):
  - kernel() sees the actual indices, so the program + table layout are
    specialized per input (compile cached on the structural shape).
  - Batch rows are assigned to cores by greedy swaps so every core
    references <= ~32.7k unique table rows; the per-core table is REMAPPED
    to exactly those rows (fp8 e4m3, premultiplied by TSCALE=64) so every
    device id fits the int16 range of `dma_gather` in a single pass —
    no two-pass zero-row-junk scheme, every descriptor fetches a needed row.
  - RUN descriptors: each (position, side) unit owns a run of RROWS=14
    consecutive table rows (its claimed rows under a hash-randomized
    first-claim plus a steal pass that tops up units short of 14; residual
    deficits duplicate the unit's own draws).  ONE descriptor with
    elem_size=14*512B / elem_step=512B (an overlapping-stride table view)
    fetches the whole run, covering 14 of the unit's 32 draws; the other 18
    are plain 512B singles sorted ascending for HBM locality (16 in per-jh
    full gathers, the last 2 slots of all 4 jh merged into one per-phase
    gather).  Descriptors drop from 65536 to 38912 per core and gather
    instructions from 64 to 40; run gathers are 512 descriptors each so
    their ring drain mostly hides under the 4-queue rotation period.
  - Accumulation over the 32 active features runs on the tensor engine with
    fp8 DoubleRow matmuls (2 row-blocks per instruction, 0.5 cyc/row)
    against a stacked identity scaled by 1/TSCALE; b_ft is added by one
    bf16 identity matmul into the same PSUM accumulation group.
  - Work is split into 16 PSUM rounds (side x batch-half x jh block, one
    bank each, double-buffered); each gather covers a single jh block, so
    every bank's DVE epilogue (clip to [0,1], multiply by W_out, reduce)
    starts as soon as its own gathers land and hides under later gathers.
"""

import sys

sys.path.insert(0, "/opt/trn_rl_repo")

import numpy as np
import ml_dtypes

from concourse import bacc, bass, mybir
import concourse.tile as tile
from concourse.bass_utils import run_bass_kernel_spmd

P = 128          # SBUF partitions
K = 32           # nnz (active features per position)
J = 8            # batch slots per partition per core
JH = 4           # j-blocks per PSUM phase
F = 512          # feature-table output width
NCORES = 8
BPC = P * J      # batch rows per core (1024)
FT_IN = 40960

BBASE = 32768            # overflow ("B") region base device row
BCAP = 576               # max overflow rows
BZ0 = 576                # B-junk ids 576..639 (zero rows)
VDEV_B = BBASE + BCAP + 64  # device rows when an overflow region is needed

f32 = mybir.dt.float32
bf16 = mybir.dt.bfloat16
i16 = mybir.dt.int16
f8 = mybir.dt.float8e4
F8_NP = ml_dtypes.float8_e4m3
BF16_NP = ml_dtypes.bfloat16
Alu = mybir.AluOpType
DR = mybir.MatmulPerfMode.DoubleRow

TSCALE = 64.0    # host premultiplier; PE identity = 1/TSCALE
NQ = 4           # SWDGE descriptor-generation queues
GBUFS = 8
NPH = 4          # phases: (side, half)
KPG = 8          # k-slots per A-gather; covers ONE jh block of 128 positions
NGJ = K // KPG   # A-gathers per (phase, jh) = 4
NGA = NGJ * JH   # A-gathers per phase = 16
NIDX = KPG * P              # idxs per A-gather (1024-descriptor HW cap)
SA16 = NIDX // 16           # 64 idx cols per A-gather

GBUFS_A = 12     # ring depth for the 4KB/partition A-gather tiles
RROWS = 14       # claimed rows per (position, side) unit (one run descriptor)
NSING = K - RROWS           # single draws per unit (18)
SING_FULL = [(0, 8), (8, 16)]   # full 1024-desc single-gathers per (ph, jh)
TAIL0 = 16       # tail slots 16..17 go in ONE per-phase gather (4 jh x 2)
NSG_P = 2        # full single-gathers per (phase, jh) in paired mode


def _build(nbslots: int, paired: bool):
    nc = bacc.Bacc("TRN2", target_bir_lowering=False, debug=False,
                   num_devices=NCORES, num_swdge_queues=NQ)

    nsg = NSG_P if paired else NGJ     # full single-gathers per (phase, jh)
    ncols = (NSG_P * JH + 1) if paired else (NGJ * JH)
    vdev = VDEV_B if nbslots else BBASE
    wft = nc.dram_tensor("w_ft", [vdev, F], f8, kind="ExternalInput")
    idxa_in = [
        nc.dram_tensor(f"idxa{ph}", [P, ncols, SA16], i16,
                       kind="ExternalInput")
        for ph in range(NPH)
    ]
    if paired:
        idxp_in = nc.dram_tensor("idxp", [P, NPH, SA16 // 2], i16,
                                 kind="ExternalInput")
    if nbslots:
        idxb_in = nc.dram_tensor("idxb", [P, NPH, JH, nbslots * 8], i16,
                                 kind="ExternalInput")
    bias_in = nc.dram_tensor("bias", [P, F], bf16, kind="ExternalInput")
    wout_in = nc.dram_tensor("wout", [P, 2, F], bf16, kind="ExternalInput")
    bout_in = nc.dram_tensor("bout", [P, 1], f32, kind="ExternalInput")
    idw_in = nc.dram_tensor("idw", [P, 2, P], f8, kind="ExternalInput")
    idb_in = nc.dram_tensor("idb", [P, P], bf16, kind="ExternalInput")
    out = nc.dram_tensor("out", [P, J], f32, kind="ExternalOutput")

    qn = 0
    with tile.TileContext(nc) as tc:
        with tc.tile_pool(name="sbuf", bufs=1) as pool, \
             tc.tile_pool(name="gather", bufs=GBUFS) as gpool, \
             tc.tile_pool(name="psum", bufs=2, space="PSUM") as ppool:
            idxa = []
            for ph in range(NPH):
                t = pool.tile([P, ncols, SA16], i16, tag=f"idxa{ph}",
                              name=f"idxa{ph}")
                idxa.append(t)
            if paired:
                idxp = pool.tile([P, NPH, SA16 // 2], i16, tag="idxp")
                nc.sync.dma_start(out=idxp[:], in_=idxp_in[:])
            # phase-0 indices first so gathers can start immediately
            nc.sync.dma_start(out=idxa[0][:], in_=idxa_in[0][:])
            identW = pool.tile([P, 2, P], f8, tag="identW")
            nc.sync.dma_start(out=identW[:], in_=idw_in[:])
            for ph in range(1, NPH):
                nc.sync.dma_start(out=idxa[ph][:], in_=idxa_in[ph][:])
            if nbslots:
                idxb = pool.tile([P, NPH, JH, nbslots * 8], i16, tag="idxb")
                nc.sync.dma_start(out=idxb[:], in_=idxb_in[:])
            bias_sb = pool.tile([P, F], bf16, tag="bias")
            nc.sync.dma_start(out=bias_sb[:], in_=bias_in[:])
            wout_sb = pool.tile([P, 2, F], bf16, tag="wout")
            nc.sync.dma_start(out=wout_sb[:], in_=wout_in[:])
            bout_sb = pool.tile([P, 1], f32, tag="bout")
            nc.sync.dma_start(out=bout_sb[:], in_=bout_in[:])
            identB = pool.tile([P, P], bf16, tag="identB")
            nc.sync.dma_start(out=identB[:], in_=idb_in[:])

            z = [pool.tile([P, J], f32, tag=f"z{s}", name=f"z{s}")
                 for s in range(2)]
            prod = pool.tile([P, F], bf16, tag="prod")

            # run-gather view: one descriptor fetches a unit's whole
            # RROWS-row claimed run (elem_step=512 scales idx by single rows)
            run_view = bass.AP(wft, 0, [[F, vdev - RROWS + 1], [1, RROWS * F]])
            gq = None
            for ph in range(NPH):
                s, hf = divmod(ph, 2)
                qn += 1    # shift rotation so run-gather drains spread queues
                if paired:
                    gq = gpool.tile([P, JH, RROWS * F], f8, tag="gq",
                                    name=f"gq{ph}", bufs=2)
                    nc.gpsimd.dma_gather(
                        gq[:], run_view, idxp[:, ph, :],
                        num_idxs=NIDX // 2, num_idxs_reg=NIDX // 2,
                        elem_size=RROWS * F, elem_step=F, queue_num=qn % NQ)
                    qn += 1
                    # tail singles for all 4 jh of this phase in one gather
                    gt = gpool.tile([P, KPG, F], f8, tag="gt",
                                    name=f"gt{ph}", bufs=2)
                    nc.gpsimd.dma_gather(
                        gt[:], wft[:, :], idxa[ph][:, NSG_P * JH, :],
                        num_idxs=NIDX, num_idxs_reg=NIDX,
                        elem_size=F, queue_num=qn % NQ)
                    qn += 1
                for jh in range(JH):
                    acc = ppool.tile([P, F], f32, tag=f"acc{jh}",
                                     name=f"acc{ph}_{jh}")
                    first = True
                    if paired:
                        rhs8 = gq[:, jh, :].rearrange(
                            "p (i f) -> p i f", i=RROWS)
                        for sl in range(RROWS // 2):
                            nc.tensor.matmul(
                                acc[:], identW[:], rhs8[:, 2 * sl:2 * sl + 2, :],
                                start=first, stop=False, perf_mode=DR)
                            first = False
                        if RROWS % 2:
                            nc.tensor.matmul(
                                acc[:], identW[:, 0, :], rhs8[:, RROWS - 1, :],
                                start=first, stop=False)
                            first = False
                    chunks = (SING_FULL if paired
                              else [(KPG * g, KPG * (g + 1))
                                    for g in range(nsg)])
                    for g, (k0, k1) in enumerate(chunks):
                        w = k1 - k0
                        ga = gpool.tile([P, w, F], f8, tag=f"ga{w}",
                                        name=f"ga{ph}_{jh}_{g}", bufs=GBUFS_A)
                        nc.gpsimd.dma_gather(
                            ga[:], wft[:, :],
                            idxa[ph][:, jh * nsg + g, 0:w * 8],
                            num_idxs=w * P, num_idxs_reg=w * P,
                            elem_size=F, queue_num=qn % NQ)
                        qn += 1
                        for kk in range(w // 2):
                            nc.tensor.matmul(
                                acc[:], identW[:],
                                ga[:, 2 * kk:2 * kk + 2, :],
                                start=first, stop=False,
                                perf_mode=DR)
                            first = False
                        if w % 2:
                            nc.tensor.matmul(
                                acc[:], identW[:, 0, :], ga[:, w - 1, :],
                                start=first, stop=False)
                            first = False
                    if paired:
                        nc.tensor.matmul(
                            acc[:], identW[:], gt[:, 2 * jh:2 * jh + 2, :],
                            start=first, stop=False, perf_mode=DR)
                        first = False
                    if nbslots:
                        gb = gpool.tile([P, nbslots, F], f8, tag=f"gb{jh}",
                                        name=f"gb{ph}_{jh}", bufs=2)
                        nc.gpsimd.dma_gather(
                            gb[:], wft[BBASE:, :], idxb[:, ph, jh, :],
                            num_idxs=nbslots * P, num_idxs_reg=nbslots * P,
                            elem_size=F, queue_num=qn % NQ)
                        qn += 1
                        for m in range(nbslots):
                            nc.tensor.matmul(
                                acc[:], identW[:, 0, :], gb[:, m, :],
                                start=False, stop=False)
                    nc.tensor.matmul(acc[:], identB[:], bias_sb[:],
                                     start=False, stop=True)

                    h = pool.tile([P, F], bf16, tag=f"h{jh}",
                                  name=f"h{ph}_{jh}")
                    nc.vector.tensor_scalar(
                        out=h[:], in0=acc[:], scalar1=0.0, scalar2=1.0,
                        op0=Alu.max, op1=Alu.min)
                    nc.vector.tensor_tensor(out=prod[:], in0=h[:],
                                            in1=wout_sb[:, s, :],
                                            op=Alu.mult)
                    nc.vector.tensor_reduce(
                        out=z[s][:, hf * JH + jh: hf * JH + jh + 1],
                        in_=prod[:], axis=mybir.AxisListType.X, op=Alu.add)

            nc.vector.tensor_tensor(out=z[0][:], in0=z[0][:], in1=z[1][:],
                                    op=Alu.add)
            out_sb = pool.tile([P, J], f32, tag="out")
            nc.scalar.activation(
                out=out_sb[:], in_=z[0][:],
                func=mybir.ActivationFunctionType.Sigmoid,
                bias=bout_sb[:, :1])
            nc.sync.dma_start(out=out.ap(), in_=out_sb[:])

    nc.compile()
    return nc


_cache = {}


def _get(nbslots: int, paired: bool):
    key = (nbslots, paired)
    if key not in _cache:
        _cache[key] = _build(nbslots, paired)
    return _cache[key]


def _wrap16(vec: np.ndarray) -> np.ndarray:
    """int vector -> [128, len//16] int16 tile (idx i at partition i%16,
    col i//16, replicated across the 8 16-partition groups)."""
    arr = vec.reshape(-1, 16).T.astype(np.int16)
    return np.tile(arr, (8, 1))


def _rebalance(stm: np.ndarray, nstm: np.ndarray):
    """Assign batch rows to cores so every core references <= 32768 unique
    table rows (then int16 gather ids cover them directly, no overflow
    pass).  Greedy pair swaps between the worst and best cores."""
    target = BBASE - 48      # headroom for paired-mode deficit duplicates
    draws = np.concatenate([stm, nstm], axis=1).astype(np.int64)  # [8192, 64]
    rows = np.arange(NCORES * BPC).reshape(NCORES, BPC)
    cnts = [np.bincount(draws[rows[c]].ravel(), minlength=FT_IN)
            for c in range(NCORES)]
    for _ in range(256):
        us = np.array([(c > 0).sum() for c in cnts])
        if us.max() <= target:
            return rows, True
        cmax, cmin = int(us.argmax()), int(us.argmin())
        dmax = draws[rows[cmax]]
        excl = (cnts[cmax][dmax] == 1).sum(axis=1)
        ra = int(excl.argmax())
        dmin = draws[rows[cmin]]
        newu = (cnts[cmax][dmin] == 0).sum(axis=1)
        rb = int(newu.argmin())
        ga, gb = rows[cmax][ra], rows[cmin][rb]
        np.subtract.at(cnts[cmax], draws[ga], 1)
        np.add.at(cnts[cmax], draws[gb], 1)
        np.subtract.at(cnts[cmin], draws[gb], 1)
        np.add.at(cnts[cmin], draws[ga], 1)
        rows[cmax][ra], rows[cmin][rb] = gb, ga
    us = np.array([(c > 0).sum() for c in cnts])
    return rows, bool(us.max() <= BBASE)


def _prep_core(stm_c: np.ndarray, nstm_c: np.ndarray, W_ft: np.ndarray,
               cap: int):
    """Remap one core's referenced table rows into the device layout and
    build its gather index vectors.

    Returns (w_dev fp8 table, avecs (NPH*NGA vectors of NIDX ids),
    bvecs (per phase: per-overflow-slot 512-id vectors), maxb)."""
    both = np.stack([stm_c, nstm_c]).astype(np.int64)   # [2, 1024, 32]
    counts = np.bincount(both.ravel(), minlength=FT_IN)
    usedrows = np.flatnonzero(counts)
    u = usedrows.size
    e = max(0, u - cap)
    if e:
        order = np.argsort(counts[usedrows], kind="stable")
        excess_rows = np.sort(usedrows[order[:e]])
        a_rows = np.sort(usedrows[order[e:]])
    else:
        excess_rows = np.empty(0, np.int64)
        a_rows = usedrows
    assert e <= BCAP, f"overflow region too small: {e} > {BCAP}"

    remap = np.zeros(FT_IN, np.int64)
    remap[a_rows] = np.arange(a_rows.size)
    remap[excess_rows] = BBASE + np.arange(e)
    dev = remap[both]                                   # [2, 1024, 32]
    dev.sort(axis=2)      # ascending: banded A ids first, overflow at tail

    vdev = VDEV_B if e else BBASE
    w_dev = np.zeros((vdev, F), dtype=F8_NP)
    w_dev[:a_rows.size] = (W_ft[a_rows] * TSCALE).astype(F8_NP)
    if e:
        w_dev[BBASE:BBASE + e] = (W_ft[excess_rows] * TSCALE).astype(F8_NP)

    spreadA = cap + (np.arange(NIDX) * 7) % 64
    spread128 = BZ0 + (np.arange(P) * 7) % 64
    avecs = []     # NPH * NGA vectors of NIDX ids, ordered (ph, jh, g)
    bvecs = []     # [ph][jh][m] -> 128-id vector (B-region-relative)
    maxb = 0
    for ph in range(NPH):
        s, hf = divmod(ph, 2)
        seg = dev[s, hf * JH * P:(hf + 1) * JH * P, :]   # [512, 32]
        segr = seg.reshape(JH, P, K)     # [jh, p, k]
        for jh in range(JH):
            for g in range(NGJ):
                # descriptor b*128+p -> (k = KPG*g + b, jh, p)
                vec = segr[jh, :, KPG * g:KPG * (g + 1)].T.ravel()
                vec = np.where(vec >= BBASE, spreadA, vec)
                avecs.append(vec)
        nB = (segr >= BBASE).sum(axis=2)                 # [jh, p]
        maxb = max(maxb, int(nB.max()))
        cols = [
            [np.where(nB[jh] > m,
                      segr[jh, np.arange(P), K - 1 - m] - BBASE, spread128)
             for m in range(int(nB.max()))]
            for jh in range(JH)
        ]
        bvecs.append(cols)
    return w_dev, avecs, bvecs, maxb


def _prep_core_paired(stm_c: np.ndarray, nstm_c: np.ndarray,
                      W_ft: np.ndarray):
    """Paired layout: each (position, side) unit owns a run of 8 table rows
    (its first-claimed rows, deficit filled by duplicating its own draws) so
    4 descriptors of elem_size=1024/elem_step=512 cover 8 of its 32 draws;
    the remaining 24 draws are plain 512B singles.

    Returns (w_dev, avecs [ph][jh*3+g] single vectors, pvecs [ph][pg] pair
    first-id vectors, ok)."""
    draws = np.stack([stm_c, nstm_c]).astype(np.int64)   # [2, 1024, 32]
    flat = draws.reshape(2 * BPC, K)                     # unit-major
    NU = 2 * BPC
    # hash-randomized claim priority so expected claims are uniform per unit
    rflat = flat.ravel()
    uflat = np.repeat(np.arange(NU), K)
    keys = (uflat * 2654435761 + rflat * 40503) % (1 << 31)
    best = np.full(FT_IN, 1 << 62, np.int64)
    np.minimum.at(best, rflat, keys)
    claim = np.full(FT_IN, NU, np.int64)
    winner = keys == best[rflat]
    np.minimum.at(claim, rflat[winner], uflat[winner])
    used = np.flatnonzero(np.bincount(rflat, minlength=FT_IN) > 0)

    RUN = RROWS
    # steal pass: units short of RUN claims take rows they draw from
    # owners holding more than RUN
    cl_count = np.bincount(claim[used], minlength=NU + 1)[:NU]
    for u in np.flatnonzero(cl_count < RUN):
        need = RUN - cl_count[u]
        for r in np.unique(flat[u]):
            if need <= 0:
                break
            o = claim[r]
            if o != u and o < NU and cl_count[o] > RUN:
                claim[r] = u
                cl_count[o] -= 1
                cl_count[u] += 1
                need -= 1

    order = np.lexsort((used, claim[used]))
    cl_sorted = used[order]
    cl_unit = claim[used][order]
    starts = np.searchsorted(cl_unit, np.arange(NU))
    ends = np.searchsorted(cl_unit, np.arange(NU) + 1)
    pair_rows = np.empty((NU, RUN), np.int64)
    remap = np.full(FT_IN, -1, np.int64)
    for u in range(NU):
        rows_u = cl_sorted[starts[u]:min(ends[u], starts[u] + RUN)]
        if rows_u.size < RUN:
            extra = np.setdiff1d(np.unique(flat[u]), rows_u)
            need = RUN - rows_u.size
            if extra.size < need:
                return None, None, None, False
            rows_u = np.concatenate([rows_u, extra[:need]])
        pair_rows[u] = rows_u
        nclaim = min(ends[u], starts[u] + RUN) - starts[u]
        remap[rows_u[:nclaim]] = u * RUN + np.arange(nclaim)
    general = used[remap[used] < 0]                      # sorted ascending
    base = NU * RUN
    if base + general.size > BBASE:
        return None, None, None, False
    remap[general] = base + np.arange(general.size)

    w_dev = np.zeros((BBASE, F), dtype=F8_NP)
    w_dev[:base] = (W_ft[pair_rows.ravel()] * TSCALE).astype(F8_NP)
    w_dev[base:base + general.size] = (
        W_ft[general] * TSCALE).astype(F8_NP)

    # per unit: the NSING single draws (draws minus one per run row)
    singles = np.empty((NU, NSING), np.int64)
    for u in range(NU):
        rows, cnts = np.unique(flat[u], return_counts=True)
        cover = np.isin(rows, pair_rows[u]).astype(np.int64)
        rest = np.repeat(rows, cnts - cover)
        if rest.size != NSING:
            return None, None, None, False
        singles[u] = np.sort(remap[rest])
    singles = singles.reshape(2, J, P, NSING)            # [s, j, p, slot]

    avecs = []
    for ph in range(NPH):
        s, hf = divmod(ph, 2)
        av = []
        for jh in range(JH):
            sl = singles[s, hf * JH + jh]                # [p, NSING]
            for k0, k1 in SING_FULL:
                av.append(sl[:, k0:k1].T.ravel())
        # combined tail gather: block b -> (jh = b//2, slot = TAIL0 + b%2)
        tail = np.empty(NIDX, np.int64)
        for b in range(KPG):
            tail[b * P:(b + 1) * P] =                 singles[s, hf * JH + b // 2, :, TAIL0 + b % 2]
        av.append(tail)
        avecs.append(av)
    # run-gather ids: desc i of phase (s, hf) covers the 8-row run of unit
    # s*BPC + hf*512 + i
    pvecs = [(s * BPC + hf * (JH * P) + np.arange(JH * P)) * RUN
             for s in (0, 1) for hf in (0, 1)]
    return w_dev, avecs, pvecs, True


def _kernel_np(stm_indices, nstm_indices, values, W_ft, b_ft, W_out, b_out):
    """Correct fallback for inputs the HW fast path doesn't cover."""
    stm_ft = np.einsum("bk,bkf->bf", values, W_ft[stm_indices]) + b_ft
    nstm_ft = np.einsum("bk,bkf->bf", values, W_ft[nstm_indices]) + b_ft
    hidden = np.clip(np.concatenate([stm_ft, nstm_ft], axis=1), 0.0, 1.0)
    return 1.0 / (1.0 + np.exp(-(hidden @ W_out + b_out)))


def kernel(stm_indices, nstm_indices, values, W_ft, b_ft, W_out, b_out,
           _trace=False):
    stm_indices = np.asarray(stm_indices)
    nstm_indices = np.asarray(nstm_indices)
    values = np.asarray(values, dtype=np.float32)
    W_ft = np.ascontiguousarray(np.asarray(W_ft, dtype=np.float32))
    b_ft = np.asarray(b_ft, dtype=np.float32)
    W_out = np.asarray(W_out, dtype=np.float32)
    b_out = np.asarray(b_out, dtype=np.float32)

    if not np.all(values == 1.0):
        r = _kernel_np(stm_indices, nstm_indices, values, W_ft, b_ft,
                       W_out, b_out)
        return (r, None) if _trace else r

    rows8, balanced = _rebalance(stm_indices, nstm_indices)
    paired = balanced
    preps = []
    nbslots = 0
    if paired:
        for c in range(NCORES):
            rc = rows8[c]
            w_dev, avecs, pvecs, ok = _prep_core_paired(
                stm_indices[rc], nstm_indices[rc], W_ft)
            if not ok:
                paired = False
                preps = []
                break
            preps.append((w_dev, avecs, pvecs))
    if not paired:
        cap = BBASE if balanced else BBASE - 64
        for c in range(NCORES):
            rc = rows8[c]
            p = _prep_core(stm_indices[rc], nstm_indices[rc], W_ft, cap)
            nbslots = max(nbslots, p[3])
            preps.append(p)

    nc = _get(nbslots, paired)
    vdev = VDEV_B if nbslots else BBASE

    bias_rep = np.ascontiguousarray(
        np.broadcast_to(b_ft, (P, F)).astype(BF16_NP))
    wout_rep = np.ascontiguousarray(
        np.broadcast_to(W_out[:, 0].reshape(2, F)[None, :, :],
                        (P, 2, F)).astype(BF16_NP))
    bout_rep = np.full((P, 1), b_out[0], dtype=np.float32)
    idw = np.zeros((P, 2, P), dtype=F8_NP)
    idw[:, 0, :] = idw[:, 1, :] = (np.eye(P) / TSCALE).astype(F8_NP)
    idb = np.ascontiguousarray(np.eye(P).astype(BF16_NP))

    in_maps = []
    for c in range(NCORES):
        if paired:
            w_dev, avecs, pvecs = preps[c]
            bvecs = None
        else:
            w_dev, avecs, bvecs, _ = preps[c]
        if w_dev.shape[0] != vdev:
            w_full = np.zeros((vdev, F), dtype=F8_NP)
            w_full[:w_dev.shape[0]] = w_dev
            w_dev = w_full
        m = {
            "w_ft": w_dev,
            "bias": bias_rep,
            "wout": wout_rep,
            "bout": bout_rep,
            "idw": idw,
            "idb": idb,
        }
        if paired:
            for ph in range(NPH):
                m[f"idxa{ph}"] = np.ascontiguousarray(np.stack(
                    [_wrap16(v) for v in avecs[ph]], axis=1))
            m["idxp"] = np.ascontiguousarray(np.stack(
                [_wrap16(v) for v in pvecs], axis=1))
        else:
            for ph in range(NPH):
                m[f"idxa{ph}"] = np.ascontiguousarray(np.stack(
                    [_wrap16(v) for v in avecs[ph * NGA:(ph + 1) * NGA]],
                    axis=1))
        if nbslots:
            idxb = np.zeros((P, NPH, JH, nbslots * 8), np.int16)
            for ph in range(NPH):
                for jh in range(JH):
                    for mi in range(nbslots):
                        if mi < len(bvecs[ph][jh]):
                            vec = bvecs[ph][jh][mi]
                        else:
                            vec = BZ0 + (np.arange(P) * 7) % 64
                        idxb[:, ph, jh, mi * 8:(mi + 1) * 8] = _wrap16(vec)
            m["idxb"] = np.ascontiguousarray(idxb)
        in_maps.append(m)

    res = run_bass_kernel_spmd(
        nc, in_maps, core_ids=list(range(NCORES)), trace=_trace
    )
    # core c's position j*128 + p holds batch row rows8[c][j*128 + p]
    out = np.zeros((NCORES * BPC, 1), dtype=np.float32)
    for c in range(NCORES):
        out[rows8[c], 0] = res.results[c]["out"].T.reshape(BPC)
    if _trace:
        return out, res
    return out
